# revision 1
# baseline (speedup 1.0000x reference)
"""Trainium2 Bass kernel for an FFM (field-aware factorization machine) forward pass.

Reference computation (all fp32):
    12 embedding matmuls over column slices of fv [32768, 2668], 15 pairwise
    dot-product cross terms, a linear layer and a sigmoid.

Restructuring: the 15 cross terms factor into 3 column-aligned block pairs
    cross = (mu+tu)·(ai+gi+oi+ui) + uu·(au+gu+ou) + mi·ti
            + au·(gu+ou) + gu·ou
so the whole model is 6 matmuls of fv @ W_block (W_block [2668, 128], built
host-side from the small tables), elementwise products of the 3 L/R pairs,
a partition-dim sum (ones-matmul), the linear term, bias and sigmoid.

Distribution: data-parallel over the batch dim — each of the 8 cores gets
4096 rows. The per-core feature matrix is transposed host-side so the device
streams [128-feature, batch] tiles straight into the PE array (contraction
dim on partitions) with no on-chip transposes. Matmuls run as float32r
(TF32-like, 1 PE cycle/row at N=512); inputs are pre-rounded to the fp32r
grid on the host, as the walrus verifier requires.
"""

import os
import numpy as np
from contextlib import ExitStack

B, F, D = 32768, 2668, 64
NCORES = 8
BL = B // NCORES          # batch rows per core
NKT = 21                  # feature K-tiles of 128
FP = NKT * 128            # padded feature dim (2688)
SUPER = 1024              # batch columns per DMA chunk
NSUB = 512                # matmul moving-dim (one fp32 PSUM bank)

BLOCK_NAMES = ("IL", "IR", "IIL", "IIR", "IIIL", "IIIR")
KTS = {
    "IL": tuple(range(7, 21)),
    "IR": tuple(range(0, 8)) + (20,),
    "IIL": tuple(range(0, 8)) + (20,),
    "IIR": (20,),
    "IIIL": (20,),
    "IIIR": (20,),
}
PAIRS = (("IL", "IR"), ("IIL", "IIR"), ("IIIL", "IIIR"))

# w_pack free-dim offsets: blocks (128 cols per K-tile), then lin (1 col per
# K-tile), then the ones column for the partition-sum reduce.
WOFF = {}
_off = 0
for _bn in BLOCK_NAMES:
    WOFF[_bn] = _off
    _off += 128 * len(KTS[_bn])
LIN_OFF = _off
_off += NKT
ONES_OFF = _off
WF = _off + 1
LIN_TILES = tuple(range(8, 20))  # t0..7 + t20 of lin ride the (IIL, IIR) pair

MM_DTYPE = os.environ.get("FFM_MM_DTYPE", "f32r")  # f32r | f32 | f16 | wf16


def _build_w_pack(inp):
    """Pack all block tables + lin_w + ones into one [128, WF] fp32 array laid
    out exactly as the SBUF weight tile wants it (partition k = row-in-K-tile)."""

    def z():
        return np.zeros((FP, D), np.float32)

    A_u, A_i = inp["age_user_w"], inp["age_item_w"]
    G_u, G_i = inp["gender_user_w"], inp["gender_item_w"]
    O_u, O_i = inp["occupation_user_w"], inp["occupation_item_w"]
    M_u, M_i = inp["movie_user_w"], inp["movie_item_w"]
    U_u, U_i = inp["userid_user_w"], inp["userid_item_w"]
    T_u, T_i = inp["itemid_user_w"], inp["itemid_item_w"]

    MT = z(); MT[943:2625] = T_u; MT[2649:2668] = M_u              # mu + tu
    TI = z(); TI[943:2625] = T_i                                    # ti
    S = z(); S[0:943] = U_i; S[2626:2627] += A_i
    S[2626:2628] += G_i; S[2628:2649] += O_i                        # ai+gi+oi+ui
    MI = z(); MI[2649:2668] = M_i                                   # mi
    UU = z(); UU[0:943] = U_u                                       # uu
    AU = z(); AU[2626:2627] = A_u                                   # au
    R = z(); R[2626:2627] += A_u; R[2626:2628] += G_u
    R[2628:2649] += O_u                                             # au+gu+ou
    GUOU = z(); GUOU[2626:2628] += G_u; GUOU[2628:2649] += O_u      # gu+ou
    GU = z(); GU[2626:2628] = G_u
    OU = z(); OU[2628:2649] = O_u
    Z = np.zeros((FP, D), np.float32)

    lw = np.zeros(FP, np.float32)
    lw[:F] = np.asarray(inp["lin_w"], np.float32)[0]
    # lin_B: the part of lin_w living in block II's K-tiles (t0..7, t20) rides
    # as column 64 of the (IIL, IIR) pair — the partner column in IIR selects
    # the host-injected ones-row (fv row 2668 == 1.0), making that product
    # column exactly lin_B. Only t8..19 keep the dedicated M=1 lin chain.
    LWB = np.zeros((FP, 1), np.float32)
    for _t in KTS["IIL"]:
        LWB[_t * 128:(_t + 1) * 128, 0] = lw[_t * 128:(_t + 1) * 128]
    E1 = np.zeros((FP, 1), np.float32)
    E1[F, 0] = 1.0  # selects the ones-feature row
    Z63 = np.zeros((FP, 63), np.float32)

    blk = {
        "IL": np.hstack([MT, TI]),
        "IR": np.hstack([S, MI]),
        "IIL": np.hstack([UU, LWB, Z63]),
        "IIR": np.hstack([R, E1, Z63]),
        "IIIL": np.hstack([GU, AU]),
        "IIIR": np.hstack([OU, GUOU]),
    }

    w_pack = np.zeros((128, WF), np.float32)
    for bn in BLOCK_NAMES:
        W = blk[bn]
        for j, t in enumerate(KTS[bn]):
            w_pack[:, WOFF[bn] + j * 128:WOFF[bn] + (j + 1) * 128] = \
                W[t * 128:(t + 1) * 128]
    for t in range(NKT):
        w_pack[:, LIN_OFF + t] = lw[t * 128:(t + 1) * 128]
    w_pack[:, ONES_OFF] = 1.0
    return w_pack


def _trace_kernel(ctx: ExitStack, tc, out_d, fvt_d, w_d, lb_d, mm_dt, w_dt,
                  onesr_d=None, repeat=1, loop=False, skip_lin=False,
                  lin_dve=False):
    import concourse.mybir as mybir

    nc = tc.nc
    f32 = mybir.dt.float32

    blocks_at_kt = [[bn for bn in BLOCK_NAMES if t in KTS[bn]]
                    for t in range(NKT)]

    wpool = ctx.enter_context(tc.tile_pool(name="wpool", bufs=1))
    w_sb = wpool.tile([128, WF], w_dt, name="w_sb")
    # Load weights hottest-first so the first matmuls aren't gated on the
    # whole 2.3 MB: the two 64 KB K-tile-0 slices of IR/IIL, then the rest of
    # the t0-needed region, then IL (first needed at K-tile 7).
    il_end = WOFF["IR"]
    for lo, hi in ((WOFF["IR"], WOFF["IR"] + 128),
                   (WOFF["IIL"], WOFF["IIL"] + 128),
                   (WOFF["IR"] + 128, WOFF["IIL"]),
                   (WOFF["IIL"] + 128, WF),
                   (0, il_end)):
        nc.sync.dma_start(w_sb[:, lo:hi], w_d[:, lo:hi])
    lb_sb = wpool.tile([1, 1], f32, name="lb_sb")
    nc.sync.dma_start(lb_sb[:], lb_d[:])

    fpool = ctx.enter_context(tc.tile_pool(name="fpool", bufs=38))
    pspool = ctx.enter_context(tc.tile_pool(name="pspool", bufs=1, space="PSUM"))
    prodpool = ctx.enter_context(tc.tile_pool(name="prodpool", bufs=3))
    opool = ctx.enter_context(tc.tile_pool(name="opool", bufs=2))

    if w_dt == mybir.dt.float16:
        # fp16 weights can't feed the f32r ones-reduce; DMA a separate f32r
        # ones vector (memset can't write f32r)
        r_dt = mybir.dt.float32r
        ones_sb = wpool.tile([128, 1], r_dt, name="ones_sb")
        nc.sync.dma_start(ones_sb[:], onesr_d[:])
        ones_ap = ones_sb[:]
    else:
        r_dt = mm_dt
        ones_ap = w_sb[:, ONES_OFF:ONES_OFF + 1]

    def _body(rep):
        for s in range(BL // SUPER):
            fts = []
            for t in range(NKT):
                ft = fpool.tile([128, SUPER], mm_dt, tag="fvt",
                                name=f"fvt_{rep}_{s}_{t}")
                # alternate the two HWDGE rings (SP / ACT) so descriptor
                # generation for the streaming loads isn't single-ring bound
                if os.environ.get("FFM_SWDGE") == "1":
                    eng = nc.gpsimd
                else:
                    eng = nc.sync if t % 2 == 0 else nc.scalar
                eng.dma_start(
                    ft[:],
                    fvt_d[t * 128:(t + 1) * 128,
                          s * SUPER:(s + 1) * SUPER])
                fts.append(ft)
            for sub in range(SUPER // NSUB):
                ps = {}
                for bn in BLOCK_NAMES:
                    ps[bn] = pspool.tile([128, NSUB], f32, tag=f"ps_{bn}",
                                         name=f"ps_{bn}_{rep}_{s}_{sub}")
                logit = pspool.tile([1, NSUB], f32, tag="logit", bufs=2,
                                    name=f"logit_{rep}_{s}_{sub}")
                accs = [None, None]  # two parities to halve the dep chain
                for t in range(NKT):
                    rhs = fts[t][:, sub * NSUB:(sub + 1) * NSUB]
                    for bn in blocks_at_kt[t]:
                        kts = KTS[bn]
                        off = WOFF[bn] + kts.index(t) * 128
                        nc.tensor.matmul(
                            ps[bn][:], w_sb[:, off:off + 128], rhs,
                            start=(t == kts[0]), stop=(t == kts[-1]))
                    if lin_dve:
                        # linear term on DVE: per-partition-scalar mult of the
                        # resident fv tile, chained accumulate in fp32
                        rhs32 = rhs.bitcast(f32)
                        w32 = w_sb[:, LIN_OFF + t:LIN_OFF + t + 1].bitcast(f32)
                        par = t % 2
                        if accs[par] is None:
                            at = prodpool.tile([128, NSUB], f32, tag=f"acc{par}",
                                               bufs=2,
                                               name=f"acc{par}_{rep}_{s}_{sub}")
                            nc.vector.tensor_single_scalar(
                                at[:], rhs32, w32, mybir.AluOpType.mult)
                            accs[par] = at
                        else:
                            nc.vector.scalar_tensor_tensor(
                                accs[par][:], rhs32, w32, accs[par][:],
                                mybir.AluOpType.mult, mybir.AluOpType.add)
                    elif not skip_lin and t in LIN_TILES:
                        nc.tensor.matmul(
                            logit[:],
                            w_sb[:, LIN_OFF + t:LIN_OFF + t + 1], rhs,
                            start=(t == LIN_TILES[0]), stop=False)
                prods = []
                for pl, pr in PAIRS:
                    # the ISA allows at most one PSUM source per
                    # tensor_tensor, so drain the L operand to SBUF first
                    lt = prodpool.tile([128, NSUB], f32, tag="ldrain",
                                       name=f"ldrain_{pl}_{rep}_{s}_{sub}")
                    if lin_dve:
                        nc.scalar.copy(lt[:], ps[pl][:])
                    else:
                        nc.vector.tensor_copy(lt[:], ps[pl][:])
                    pt = prodpool.tile([128, NSUB], r_dt, tag="prod", bufs=4,
                                       name=f"prod_{pl}_{rep}_{s}_{sub}")
                    nc.vector.tensor_mul(pt[:], lt[:], ps[pr][:])
                    prods.append(pt)
                if lin_dve:
                    p4 = prodpool.tile([128, NSUB], r_dt, tag="prod", bufs=4,
                                       name=f"prod_lin_{rep}_{s}_{sub}")
                    nc.vector.tensor_add(p4[:], accs[0][:], accs[1][:])
                    prods.append(p4)
                first_start = skip_lin or lin_dve
                for j, pt in enumerate(prods):
                    nc.tensor.matmul(logit[:], ones_ap, pt[:],
                                     start=(first_start and j == 0),
                                     stop=(j == len(prods) - 1))
                out_sb = opool.tile([1, NSUB], f32, tag="out",
                                    name=f"out_{rep}_{s}_{sub}")
                nc.scalar.activation(out_sb[:], logit[:],
                                     mybir.ActivationFunctionType.Sigmoid,
                                     bias=lb_sb[0:1, 0:1], scale=1.0)
                col = s * SUPER + sub * NSUB
                nc.scalar.dma_start(out_d[0:1, col:col + NSUB], out_sb[:])

    if loop and repeat > 1:
        # benchmarking mode: run the identical body `repeat` times inside one
        # NEFF via a hardware loop (one dispatch, `repeat` full passes)
        with tc.For_i(0, repeat, 1):
            _body(0)
    else:
        for rep in range(repeat):
            _body(rep)


_MODULES = {}


def get_module(repeat=1, loop=False, skip_lin=False, lin_dve=False):
    """Build (once per config) and return the compiled Bass module."""
    key = (repeat, loop, skip_lin, lin_dve)
    if key in _MODULES:
        return _MODULES[key]

    import concourse.bacc as bacc
    import concourse.tile as tile
    import concourse.mybir as mybir

    mm_dt = {"f32r": mybir.dt.float32r, "f32": mybir.dt.float32,
             "f16": mybir.dt.float16, "wf16": mybir.dt.float32r}[MM_DTYPE]
    w_dt = mybir.dt.float16 if MM_DTYPE in ("f16", "wf16") else mm_dt

    nc = bacc.Bacc("TRN2", debug=False, enable_asserts=False,
                   num_devices=NCORES)
    fvt_d = nc.dram_tensor("fvt", (FP, BL), mm_dt,
                           kind="ExternalInput").ap()
    w_d = nc.dram_tensor("wpack", (128, WF), w_dt,
                         kind="ExternalInput").ap()
    lb_d = nc.dram_tensor("linb", (1, 1), mybir.dt.float32,
                          kind="ExternalInput").ap()
    onesr_d = None
    if MM_DTYPE in ("f16", "wf16"):
        onesr_d = nc.dram_tensor("onesr", (128, 1), mybir.dt.float32r,
                                 kind="ExternalInput").ap()
    out_d = nc.dram_tensor("out", (1, BL), mybir.dt.float32,
                           kind="ExternalOutput").ap()

    with tile.TileContext(nc) as tc, ExitStack() as ctx:
        _trace_kernel(ctx, tc, out_d, fvt_d, w_d, lb_d, mm_dt, w_dt,
                      onesr_d=onesr_d, repeat=repeat, loop=loop,
                      skip_lin=skip_lin, lin_dve=lin_dve)
    nc.compile()
    _MODULES[key] = nc
    return nc


def _to_f32r(x):
    from neuron_dtypes import static_cast_fp32_to_fp32r
    return np.ascontiguousarray(
        static_cast_fp32_to_fp32r(np.ascontiguousarray(x))
    ).view(np.float32).reshape(x.shape)


def _round_fv(x):
    if MM_DTYPE == "f16":
        return np.ascontiguousarray(x, np.float16)
    if MM_DTYPE in ("f32r", "wf16"):
        return _to_f32r(x)
    return x


def _round_w(x):
    if MM_DTYPE in ("f16", "wf16"):
        return np.ascontiguousarray(x, np.float16)
    if MM_DTYPE == "f32r":
        return _to_f32r(x)
    return x


def prepare_in_maps(inputs):
    """Host-side sharding: batch-split fv, transpose each shard to
    feature-major (padded to 2688 rows), replicate the packed weights."""
    fv = np.ascontiguousarray(np.asarray(inputs["feature_vector"], np.float32))
    assert fv.shape == (B, F)
    w_pack = _round_w(_build_w_pack({k: np.asarray(v, np.float32)
                                     for k, v in inputs.items()
                                     if k != "feature_vector"}))
    lb = np.asarray(inputs["lin_b"], np.float32).reshape(1, 1)

    in_maps = []
    for c in range(NCORES):
        fvt = np.zeros((FP, BL), np.float32)
        fvt[:F] = fv[c * BL:(c + 1) * BL].T
        fvt[F] = 1.0  # ones-feature row pairing with lin_B in block II
        m = {"fvt": _round_fv(fvt), "wpack": w_pack, "linb": lb}
        if MM_DTYPE in ("f16", "wf16"):
            m["onesr"] = np.ones((128, 1), np.float32)
        in_maps.append(m)
    return in_maps


def kernel(**inputs) -> np.ndarray:
    # Tracing needs the axon NTFF hook, which this environment lacks; make
    # sure a stray BASS_TRACE=1 can't crash the run.
    os.environ["BASS_NEVER_TRACE"] = "1"
    from concourse import bass_utils

    in_maps = prepare_in_maps(inputs)
    nc = get_module()
    try:
        res = bass_utils.run_bass_kernel_spmd(nc, in_maps,
                                              core_ids=list(range(NCORES)))
    except Exception:
        # transient NRT device errors have been observed on this fabric;
        # one retry after a short pause usually succeeds
        import time
        time.sleep(15)
        res = bass_utils.run_bass_kernel_spmd(nc, in_maps,
                                              core_ids=list(range(NCORES)))
    out = np.concatenate([r["out"].reshape(BL) for r in res.results])
    return out.reshape(B, 1).astype(np.float32)



# revision 22
# speedup vs baseline: 1.2371x; 1.2371x over previous
"""Trainium2 Bass kernel for an FFM (field-aware factorization machine) forward pass.

Reference computation (all fp32):
    12 embedding matmuls over column slices of fv [32768, 2668], 15 pairwise
    dot-product cross terms, a linear layer and a sigmoid.

Restructuring: the 15 cross terms factor into 3 column-aligned block pairs
    cross = (mu+tu)·(ai+gi+oi+ui) + uu·(au+gu+ou) + mi·ti
            + au·(gu+ou) + gu·ou
so the whole model is 6 matmuls of fv @ W_block (W_block [2668, 128], built
host-side from the small tables), elementwise products of the 3 L/R pairs,
a partition-dim sum (ones-matmul), the linear term, bias and sigmoid.

Distribution: data-parallel over the batch dim — each of the 8 cores gets
4096 rows. The per-core feature matrix is transposed and packed host-side in
fp16 (halving the HBM stream, which is the roofline for this kernel) in a
[128, supers, K-tiles, cols] layout so one DMA per half-super moves 21
K-tiles as fully contiguous 20-22 KB per-partition runs.

Engine split per 512-column chunk (v2): 35 block matmuls + 2 ones-reduce
matmuls on PE; the linear term for K-tiles 8..19 as per-partition-scalar
MACs on DVE (8) and GpSimd (4); the lin part for tiles 0..7+20 rides column
64 of the (IIL, IIR) pair product against a host-injected ones feature; PSUM
drains on ACT; pair products + merge adds on DVE.
"""

import os
import numpy as np
from contextlib import ExitStack

B, F, D = 32768, 2668, 64
NCORES = 8
BL = B // NCORES          # batch rows per core
NKT = 21                  # feature K-tiles of 128
FP = NKT * 128            # padded feature dim (2688)
SUPER = int(os.environ.get("FFM_SUPER", "512"))  # batch cols per DMA chunk
NSUB = 512                # matmul moving-dim (one fp32 PSUM bank)
NSUPER = BL // SUPER
NKT_A = 11                # K-tiles in the first half-super DMA (ring A)
NKT_B = NKT - NKT_A

BLOCK_NAMES = ("IL", "IR", "IIL", "IIR", "IIIL", "IIIR")
KTS = {
    "IL": tuple(range(7, 21)),
    "IR": tuple(range(0, 8)) + (20,),
    "IIL": tuple(range(0, 8)) + (20,),
    "IIR": (20,),
    "IIIL": (20,),
    "IIIR": (20,),
}
PAIRS = (("IL", "IR"), ("IIL", "IIR"), ("IIIL", "IIIR"))

# w_pack free-dim offsets: blocks (128 cols per K-tile), then lin (1 col per
# K-tile), then the ones column for the partition-sum reduce.
WOFF = {}
_off = 0
for _bn in BLOCK_NAMES:
    WOFF[_bn] = _off
    _off += 128 * len(KTS[_bn])
LIN_OFF = _off
_off += NKT
ONES_OFF = _off
WF = _off + 1
LIN_TILES = tuple(range(8, 20))  # t8..19: lin handled off-PE (v2) / M=1 (v1)

MM_DTYPE = os.environ.get("FFM_MM_DTYPE", "f16")   # f16 | f32r | f32
KERNEL_V = int(os.environ.get("FFM_V", "2"))
POOL_MACS = int(os.environ.get("FFM_POOL_MACS", "6"))
ACC16 = os.environ.get("FFM_ACC16", "1") == "1"
RED = int(os.environ.get("FFM_RED", "2"))          # ones-reduce matmuls (1|2|3)


def _build_w_pack(inp):
    """Pack all block tables + lin_w + ones into one [128, WF] fp32 array laid
    out exactly as the SBUF weight tile wants it (partition k = row-in-K-tile)."""

    def z():
        return np.zeros((FP, D), np.float32)

    A_u, A_i = inp["age_user_w"], inp["age_item_w"]
    G_u, G_i = inp["gender_user_w"], inp["gender_item_w"]
    O_u, O_i = inp["occupation_user_w"], inp["occupation_item_w"]
    M_u, M_i = inp["movie_user_w"], inp["movie_item_w"]
    U_u, U_i = inp["userid_user_w"], inp["userid_item_w"]
    T_u, T_i = inp["itemid_user_w"], inp["itemid_item_w"]

    MT = z(); MT[943:2625] = T_u; MT[2649:2668] = M_u              # mu + tu
    TI = z(); TI[943:2625] = T_i                                    # ti
    S = z(); S[0:943] = U_i; S[2626:2627] += A_i
    S[2626:2628] += G_i; S[2628:2649] += O_i                        # ai+gi+oi+ui
    MI = z(); MI[2649:2668] = M_i                                   # mi
    UU = z(); UU[0:943] = U_u                                       # uu
    AU = z(); AU[2626:2627] = A_u                                   # au
    R = z(); R[2626:2627] += A_u; R[2626:2628] += G_u
    R[2628:2649] += O_u                                             # au+gu+ou
    GUOU = z(); GUOU[2626:2628] += G_u; GUOU[2628:2649] += O_u      # gu+ou
    GU = z(); GU[2626:2628] = G_u
    OU = z(); OU[2628:2649] = O_u

    lw = np.zeros(FP, np.float32)
    lw[:F] = np.asarray(inp["lin_w"], np.float32)[0]
    # lin_B: the part of lin_w living in block II's K-tiles (t0..7, t20) rides
    # as column 64 of the (IIL, IIR) pair — the partner column in IIR selects
    # the host-injected ones-row (fv row 2668 == 1.0), making that product
    # column exactly lin_B. Only t8..19 need separate handling.
    LWB = np.zeros((FP, 1), np.float32)
    for _t in KTS["IIL"]:
        LWB[_t * 128:(_t + 1) * 128, 0] = lw[_t * 128:(_t + 1) * 128]
    E1 = np.zeros((FP, 1), np.float32)
    E1[F, 0] = 1.0  # selects the ones-feature row
    Z63 = np.zeros((FP, 63), np.float32)

    blk = {
        "IL": np.hstack([MT, TI]),
        "IR": np.hstack([S, MI]),
        "IIL": np.hstack([UU, LWB, Z63]),
        "IIR": np.hstack([R, E1, Z63]),
        "IIIL": np.hstack([GU, AU]),
        "IIIR": np.hstack([OU, GUOU]),
    }

    w_pack = np.zeros((128, WF), np.float32)
    for bn in BLOCK_NAMES:
        W = blk[bn]
        for j, t in enumerate(KTS[bn]):
            w_pack[:, WOFF[bn] + j * 128:WOFF[bn] + (j + 1) * 128] = \
                W[t * 128:(t + 1) * 128]
    for t in range(NKT):
        w_pack[:, LIN_OFF + t] = lw[t * 128:(t + 1) * 128]
    w_pack[:, ONES_OFF] = 1.0
    return w_pack


def _trace_kernel_v2(ctx: ExitStack, tc, out_d, fvt_d, w_d, lb_d, onesr_d,
                     linw_d, mm_dt, repeat=1, loop=False):
    import concourse.mybir as mybir

    nc = tc.nc
    f32 = mybir.dt.float32
    f32r = mybir.dt.float32r
    acc_dt = mybir.dt.float16 if ACC16 else f32
    mult, add = mybir.AluOpType.mult, mybir.AluOpType.add

    # K-tile issue order: userid tiles, then t20 (completing IR/IIL/IIR/
    # IIIL/IIIR mid-chunk so their products run early), then itemid tiles.
    # Only pair 1 (IL x IR) completes at the end of the chunk.
    ORDER = tuple(range(0, 8)) + (20,) + tuple(range(8, 20))
    OPOS = {t: i for i, t in enumerate(ORDER)}
    blocks_at_kt = [[bn for bn in BLOCK_NAMES if t in KTS[bn]]
                    for t in range(NKT)]
    first_last = {}
    for bn in BLOCK_NAMES:
        kts = sorted(KTS[bn], key=lambda t: OPOS[t])
        first_last[bn] = (kts[0], kts[-1])

    wpool = ctx.enter_context(tc.tile_pool(name="wpool", bufs=1))
    w_sb = wpool.tile([128, WF], mm_dt, name="w_sb")
    nc.sync.dma_start(w_sb[:], w_d[:])
    lb_sb = wpool.tile([1, 1], f32, name="lb_sb")
    nc.sync.dma_start(lb_sb[:], lb_d[:])
    ones_sb = wpool.tile([128, 1], f32r, name="ones_sb")
    nc.scalar.dma_start(ones_sb[:], onesr_d[:])
    linw_sb = wpool.tile([128, NKT], f32, name="linw_sb")
    nc.scalar.dma_start(linw_sb[:], linw_d[:])

    fpool = ctx.enter_context(
        tc.tile_pool(name="fpool", bufs=int(os.environ.get("FFM_FBUFS", "4"))))
    pspool = ctx.enter_context(tc.tile_pool(name="pspool", bufs=1, space="PSUM"))
    tpool = ctx.enter_context(tc.tile_pool(name="tpool", bufs=2))
    opool = ctx.enter_context(tc.tile_pool(name="opool", bufs=2))

    def _body(rep):
        out_sb = opool.tile([1, BL], f32, tag="out", name=f"out_{rep}")
        pending = [None]  # deferred (reduce sA + sigmoid) of previous chunk

        def flush_pending():
            if pending[0] is not None:
                pending[0]()
                pending[0] = None

        for s in range(NSUPER):
            fta = fpool.tile([128, NKT_A * SUPER], mm_dt, tag="fta",
                             name=f"fta_{rep}_{s}")
            ftb = fpool.tile([128, NKT_B * SUPER], mm_dt, tag="ftb",
                             name=f"ftb_{rep}_{s}")
            # both streaming DMAs ride the SP ring: it carries no engine
            # work, so descriptor generation is never queued behind drains
            # or sigmoids the way the ACT ring's would be
            base = s * NKT * SUPER
            nc.sync.dma_start(
                fta[:], fvt_d[:, base:base + NKT_A * SUPER])
            nc.sync.dma_start(
                ftb[:], fvt_d[:, base + NKT_A * SUPER:base + NKT * SUPER])

            def rhs_of(t, sub):
                col = t * SUPER + sub * NSUB if t < NKT_A else \
                    (t - NKT_A) * SUPER + sub * NSUB
                ft = fta if t < NKT_A else ftb
                return ft[:, col:col + NSUB]

            for sub in range(SUPER // NSUB):
                sfx = f"{rep}_{s}_{sub}"
                ps = {bn: pspool.tile([128, NSUB], f32, tag=f"ps_{bn}",
                                      name=f"ps_{bn}_{sfx}")
                      for bn in BLOCK_NAMES}
                logit = pspool.tile([1, NSUB], f32, tag="logit", bufs=2,
                                    name=f"logit_{sfx}")
                accv = [None, None]
                nv = 0
                prods = {}

                def lin_mac(t, rhs):
                    # walrus rejects TensorScalarPtr on Pool, so every lin
                    # MAC lives on DVE; Pool instead owns the SBUF-only
                    # products and merges below.
                    nonlocal nv
                    w_col = linw_sb[:, t:t + 1]
                    par = nv % 2
                    nv += 1
                    if accv[par] is None:
                        at = tpool.tile([128, NSUB], acc_dt,
                                        tag=f"accv{par}",
                                        name=f"accv{par}_{sfx}")
                        nc.vector.tensor_single_scalar(
                            at[:], rhs, w_col, mult)
                        accv[par] = at
                    else:
                        nc.vector.scalar_tensor_tensor(
                            accv[par][:], rhs, w_col, accv[par][:],
                            mult, add)

                def pair_prod(pl, pr):
                    # ACT drains both psums to SBUF; Pool multiplies them
                    # (gpsimd has no PSUM port, so it only sees SBUF tiles).
                    rt = tpool.tile([128, NSUB], f32, tag=f"rt_{pr}",
                                    name=f"rt_{pr}_{sfx}")
                    nc.scalar.copy(rt[:], ps[pr][:])
                    lt = tpool.tile([128, NSUB], f32, tag=f"lt_{pl}",
                                    name=f"lt_{pl}_{sfx}")
                    nc.scalar.copy(lt[:], ps[pl][:])
                    pt = tpool.tile([128, NSUB], f32, tag=f"p_{pl}",
                                    name=f"p_{pl}_{sfx}")
                    nc.gpsimd.tensor_mul(pt[:], rt[:], lt[:])
                    prods[pl] = pt

                for i, t in enumerate(ORDER):
                    if i == 5:
                        flush_pending()
                    rhs = rhs_of(t, sub)
                    for bn in blocks_at_kt[t]:
                        kts = KTS[bn]
                        off = WOFF[bn] + kts.index(t) * 128
                        fl = first_last[bn]
                        nc.tensor.matmul(
                            ps[bn][:], w_sb[:, off:off + 128], rhs,
                            start=(t == fl[0]), stop=(t == fl[1]))
                    if t == 20:
                        # IR, IIL, IIR, IIIL, IIIR just completed
                        pair_prod("IIL", "IIR")
                        pair_prod("IIIL", "IIIR")
                        rt1 = tpool.tile([128, NSUB], f32, tag="rt_IR",
                                         name=f"rt_IR_{sfx}")
                        nc.scalar.copy(rt1[:], ps["IR"][:])
                    if t in LIN_TILES:
                        lin_mac(t, rhs)

                # after t19: IL complete. Pool pre-merged the early products
                # (m1 = p2 + p3); DVE finishes pair 1 (f32r out, feeding the
                # ones-matmul directly) and folds the lin accumulators into
                # m1. Both ones-reduces + the sigmoid are deferred into the
                # next chunk's PE stream so the in-order PE queue never
                # waits on them.
                m1 = tpool.tile([128, NSUB], f32, tag="m1", name=f"m1_{sfx}")
                nc.gpsimd.tensor_add(m1[:], prods["IIL"][:],
                                     prods["IIIL"][:])
                va = tpool.tile([128, NSUB], f32, tag="va", name=f"va_{sfx}")
                nc.vector.tensor_add(va[:], accv[0][:], accv[1][:])
                sB = tpool.tile([128, NSUB], f32r, tag="sB", name=f"sB_{sfx}")
                if os.environ.get("FFM_SB_POOL", "1") == "1":
                    nc.gpsimd.tensor_add(sB[:], m1[:], va[:])
                else:
                    nc.vector.tensor_add(sB[:], m1[:], va[:])
                p1 = tpool.tile([128, NSUB], f32r, tag="p_IL",
                                name=f"p_IL_{sfx}")
                nc.vector.tensor_mul(p1[:], rt1[:], ps["IL"][:])

                col = s * SUPER + sub * NSUB

                if RED == 1:
                    sT = tpool.tile([128, NSUB], f32r, tag="sT",
                                    name=f"sT_{sfx}")
                    nc.vector.tensor_add(sT[:], p1[:].bitcast(f32),
                                         sB[:].bitcast(f32))

                    def tail(logit=logit, sT=sT, col=col):
                        nc.tensor.matmul(logit[:], ones_sb[:],
                                         sT[:],
                                         start=True, stop=True)
                        nc.scalar.activation(
                            out_sb[0:1, col:col + NSUB], logit[:],
                            mybir.ActivationFunctionType.Sigmoid,
                            bias=lb_sb[0:1, 0:1], scale=1.0)
                else:
                    def tail(logit=logit, p1=p1, sB=sB, col=col):
                        nc.tensor.matmul(logit[:], ones_sb[:],
                                         sB[:],
                                         start=True, stop=False)
                        nc.tensor.matmul(logit[:], ones_sb[:],
                                         p1[:],
                                         start=False, stop=True)
                        nc.scalar.activation(
                            out_sb[0:1, col:col + NSUB], logit[:],
                            mybir.ActivationFunctionType.Sigmoid,
                            bias=lb_sb[0:1, 0:1], scale=1.0)

                pending[0] = tail
        flush_pending()
        nc.scalar.dma_start(out_d[:], out_sb[:])

    if loop and repeat > 1:
        with tc.For_i(0, repeat, 1):
            _body(0)
    else:
        for rep in range(repeat):
            _body(rep)


def _trace_kernel_v1(ctx: ExitStack, tc, out_d, fvt_d, w_d, lb_d, mm_dt, w_dt,
                     onesr_d=None, repeat=1, loop=False):
    """Previous-generation body: per-K-tile DMAs, lin via M=1 PE chain."""
    import concourse.mybir as mybir

    nc = tc.nc
    f32 = mybir.dt.float32

    blocks_at_kt = [[bn for bn in BLOCK_NAMES if t in KTS[bn]]
                    for t in range(NKT)]

    wpool = ctx.enter_context(tc.tile_pool(name="wpool", bufs=1))
    w_sb = wpool.tile([128, WF], w_dt, name="w_sb")
    il_end = WOFF["IR"]
    for lo, hi in ((WOFF["IR"], WOFF["IR"] + 128),
                   (WOFF["IIL"], WOFF["IIL"] + 128),
                   (WOFF["IR"] + 128, WOFF["IIL"]),
                   (WOFF["IIL"] + 128, WF),
                   (0, il_end)):
        nc.sync.dma_start(w_sb[:, lo:hi], w_d[:, lo:hi])
    lb_sb = wpool.tile([1, 1], f32, name="lb_sb")
    nc.sync.dma_start(lb_sb[:], lb_d[:])

    fpool = ctx.enter_context(tc.tile_pool(name="fpool", bufs=38))
    pspool = ctx.enter_context(tc.tile_pool(name="pspool", bufs=1, space="PSUM"))
    prodpool = ctx.enter_context(tc.tile_pool(name="prodpool", bufs=3))
    opool = ctx.enter_context(tc.tile_pool(name="opool", bufs=2))

    if w_dt == mybir.dt.float16:
        r_dt = mybir.dt.float32r
        ones_sb = wpool.tile([128, 1], r_dt, name="ones_sb")
        nc.sync.dma_start(ones_sb[:], onesr_d[:])
        ones_ap = ones_sb[:]
    else:
        r_dt = mm_dt
        ones_ap = w_sb[:, ONES_OFF:ONES_OFF + 1]

    def _body(rep):
        for s in range(BL // SUPER):
            fts = []
            for t in range(NKT):
                ft = fpool.tile([128, SUPER], mm_dt, tag="fvt",
                                name=f"fvt_{rep}_{s}_{t}")
                eng = nc.sync if t % 2 == 0 else nc.scalar
                eng.dma_start(
                    ft[:],
                    fvt_d[t * 128:(t + 1) * 128,
                          s * SUPER:(s + 1) * SUPER])
                fts.append(ft)
            for sub in range(SUPER // NSUB):
                ps = {}
                for bn in BLOCK_NAMES:
                    ps[bn] = pspool.tile([128, NSUB], f32, tag=f"ps_{bn}",
                                         name=f"ps_{bn}_{rep}_{s}_{sub}")
                logit = pspool.tile([1, NSUB], f32, tag="logit", bufs=2,
                                    name=f"logit_{rep}_{s}_{sub}")
                for t in range(NKT):
                    rhs = fts[t][:, sub * NSUB:(sub + 1) * NSUB]
                    for bn in blocks_at_kt[t]:
                        kts = KTS[bn]
                        off = WOFF[bn] + kts.index(t) * 128
                        nc.tensor.matmul(
                            ps[bn][:], w_sb[:, off:off + 128], rhs,
                            start=(t == kts[0]), stop=(t == kts[-1]))
                    if t in LIN_TILES:
                        nc.tensor.matmul(
                            logit[:],
                            w_sb[:, LIN_OFF + t:LIN_OFF + t + 1], rhs,
                            start=(t == LIN_TILES[0]), stop=False)
                prods = []
                for pl, pr in PAIRS:
                    lt = prodpool.tile([128, NSUB], f32, tag="ldrain",
                                       name=f"ldrain_{pl}_{rep}_{s}_{sub}")
                    nc.vector.tensor_copy(lt[:], ps[pl][:])
                    pt = prodpool.tile([128, NSUB], r_dt, tag="prod", bufs=4,
                                       name=f"prod_{pl}_{rep}_{s}_{sub}")
                    nc.vector.tensor_mul(pt[:], lt[:], ps[pr][:])
                    prods.append(pt)
                for j, pt in enumerate(prods):
                    nc.tensor.matmul(logit[:], ones_ap, pt[:],
                                     start=False,
                                     stop=(j == len(prods) - 1))
                out_sb = opool.tile([1, NSUB], f32, tag="out",
                                    name=f"out_{rep}_{s}_{sub}")
                nc.scalar.activation(out_sb[:], logit[:],
                                     mybir.ActivationFunctionType.Sigmoid,
                                     bias=lb_sb[0:1, 0:1], scale=1.0)
                col = s * SUPER + sub * NSUB
                nc.scalar.dma_start(out_d[0:1, col:col + NSUB], out_sb[:])

    if loop and repeat > 1:
        with tc.For_i(0, repeat, 1):
            _body(0)
    else:
        for rep in range(repeat):
            _body(rep)


_MODULES = {}


def get_module(repeat=1, loop=False, skip_lin=False, lin_dve=False):
    """Build (once per config) and return the compiled Bass module."""
    key = (repeat, loop, KERNEL_V)
    if key in _MODULES:
        return _MODULES[key]

    import concourse.bacc as bacc
    import concourse.tile as tile
    import concourse.mybir as mybir

    mm_dt = {"f32r": mybir.dt.float32r, "f32": mybir.dt.float32,
             "f16": mybir.dt.float16}[MM_DTYPE]
    w_dt = mm_dt

    nc = bacc.Bacc("TRN2", debug=False, enable_asserts=False,
                   num_devices=NCORES)
    lb_d = nc.dram_tensor("linb", (1, 1), mybir.dt.float32,
                          kind="ExternalInput").ap()
    out_d = nc.dram_tensor("out", (1, BL), mybir.dt.float32,
                           kind="ExternalOutput").ap()
    w_d = nc.dram_tensor("wpack", (128, WF), w_dt,
                         kind="ExternalInput").ap()

    if KERNEL_V == 2:
        fvt_d = nc.dram_tensor("fvt", (128, NKT * BL), mm_dt,
                               kind="ExternalInput").ap()
        onesr_d = nc.dram_tensor("onesr", (128, 1), mybir.dt.float32r,
                                 kind="ExternalInput").ap()
        linw_d = nc.dram_tensor("linw", (128, NKT), mybir.dt.float32,
                                kind="ExternalInput").ap()
        with tile.TileContext(nc) as tc, ExitStack() as ctx:
            _trace_kernel_v2(ctx, tc, out_d, fvt_d, w_d, lb_d, onesr_d,
                             linw_d, mm_dt, repeat=repeat, loop=loop)
    else:
        fvt_d = nc.dram_tensor("fvt", (FP, BL), mm_dt,
                               kind="ExternalInput").ap()
        onesr_d = None
        if MM_DTYPE == "f16":
            onesr_d = nc.dram_tensor("onesr", (128, 1), mybir.dt.float32r,
                                     kind="ExternalInput").ap()
        with tile.TileContext(nc) as tc, ExitStack() as ctx:
            _trace_kernel_v1(ctx, tc, out_d, fvt_d, w_d, lb_d, mm_dt, w_dt,
                             onesr_d=onesr_d, repeat=repeat, loop=loop)
    nc.compile()
    _MODULES[key] = nc
    return nc


def _to_f32r(x):
    from neuron_dtypes import static_cast_fp32_to_fp32r
    return np.ascontiguousarray(
        static_cast_fp32_to_fp32r(np.ascontiguousarray(x))
    ).view(np.float32).reshape(x.shape)


def _round_fv(x):
    if MM_DTYPE == "f16":
        return np.ascontiguousarray(x, np.float16)
    if MM_DTYPE == "f32r":
        return _to_f32r(x)
    return np.ascontiguousarray(x, np.float32)


def prepare_in_maps(inputs):
    """Host-side sharding: batch-split fv, transpose each shard to
    feature-major (padded to 2688 rows), replicate the packed weights."""
    fv = np.ascontiguousarray(np.asarray(inputs["feature_vector"], np.float32))
    assert fv.shape == (B, F)
    w_pack = _round_fv(_build_w_pack({k: np.asarray(v, np.float32)
                                      for k, v in inputs.items()
                                      if k != "feature_vector"}))
    lb = np.asarray(inputs["lin_b"], np.float32).reshape(1, 1)

    in_maps = []
    for c in range(NCORES):
        fvt = np.zeros((FP, BL), np.float32)
        fvt[:F] = fv[c * BL:(c + 1) * BL].T
        fvt[F] = 1.0  # ones-feature row pairing with lin_B in block II
        fvt = _round_fv(fvt)
        if KERNEL_V == 2:
            # [p, s, t, c] layout: per-partition contiguous 21*SUPER runs
            fvt = np.ascontiguousarray(
                fvt.reshape(NKT, 128, NSUPER, SUPER).transpose(1, 2, 0, 3)
            ).reshape(128, NKT * BL)
        m = {"fvt": fvt, "wpack": w_pack, "linb": lb}
        if KERNEL_V == 2 or MM_DTYPE == "f16":
            m["onesr"] = np.ones((128, 1), np.float32)
        if KERNEL_V == 2:
            lw = np.zeros(FP, np.float32)
            lw[:F] = np.asarray(inputs["lin_w"], np.float32)[0]
            m["linw"] = np.ascontiguousarray(
                lw.reshape(NKT, 128).T, np.float32)
        in_maps.append(m)
    return in_maps


def kernel(**inputs) -> np.ndarray:
    # Tracing needs the axon NTFF hook, which this environment lacks; make
    # sure a stray BASS_TRACE=1 can't crash the run.
    os.environ["BASS_NEVER_TRACE"] = "1"
    from concourse import bass_utils

    in_maps = prepare_in_maps(inputs)
    nc = get_module()
    try:
        res = bass_utils.run_bass_kernel_spmd(nc, in_maps,
                                              core_ids=list(range(NCORES)))
    except Exception:
        # transient NRT device errors have been observed on this fabric;
        # one retry after a short pause usually succeeds
        import time
        time.sleep(15)
        res = bass_utils.run_bass_kernel_spmd(nc, in_maps,
                                              core_ids=list(range(NCORES)))
    out = np.concatenate([r["out"].reshape(BL) for r in res.results])
    return out.reshape(B, 1).astype(np.float32)
